# revision 1
# baseline (speedup 1.0000x reference)
"""TRN2 Bass/Tile kernel for nn_Block_19756849561899 (pre-LN transformer
block: LN -> MHA -> residual -> LN -> MLP(gelu) -> residual).

Self-contained: kernel(**inputs) takes the full fp32 tensors, shards work
across 8 NeuronCores (one batch per core-pair; each core owns half the
sequence as queries and redundantly builds K/V for its batch), compiles a
Bass/Tile program once per process, runs it SPMD, and reassembles the full
output.
"""

import contextlib

import numpy as np
import ml_dtypes

import concourse.bass as bass
import concourse.mybir as mybir
import concourse.tile as tile
from concourse.masks import make_identity

fp32 = mybir.dt.float32
bf16 = mybir.dt.bfloat16
fp8 = mybir.dt.float8e4
AF = mybir.ActivationFunctionType
ALU = mybir.AluOpType
AX = mybir.AxisListType

C = 384
CS = 3          # C / 128
H = 6
HP = 3          # head pairs
DH = 64
HID = 1536
KS = 12         # HID / 128
VW = 72         # padded V row width (DoubleRow needs 16B-aligned pair stride)
WSCALE = 16.0   # fp8 MLP weight scaling (avoids e4m3 denormals)
EPS = 1e-6
NBIAS = 24


def build(nc, SEQ=2048, act_fn=AF.Gelu):
    TT = SEQ // 128          # token tiles over full sequence
    QTT = TT // 2            # token tiles in own (query) half
    QLEN = SEQ // 2
    QF = min(512, QLEN)      # q free-dim tile
    NJ = QLEN // QF
    NF = min(512, SEQ)       # seq free-dim tile for K^T build
    NN = SEQ // NF
    NB = QF // 128           # token blocks per q-tile
    CK = 2                   # key tiles per S/exp chunk
    chunks = [(k0, min(CK, TT - k0)) for k0 in range(0, TT, CK)]

    xin = nc.dram_tensor("xin", [SEQ, C], fp32, kind="ExternalInput")
    wqk_d = nc.dram_tensor("wqk", [128, CS, 768], bf16, kind="ExternalInput")
    wv_d = nc.dram_tensor("wv", [128, CS, C], bf16, kind="ExternalInput")
    wp_d = nc.dram_tensor("wp", [128, CS, C], bf16, kind="ExternalInput")
    wf1_d = nc.dram_tensor("wf1", [128, CS, HID], bf16, kind="ExternalInput")
    wf2_d = nc.dram_tensor("wf2", [128, KS, C], bf16, kind="ExternalInput")
    bias_d = nc.dram_tensor("bias", [128, NBIAS], fp32, kind="ExternalInput")
    bv_d = nc.dram_tensor("bv", [1, C], fp32, kind="ExternalInput")
    yout = nc.dram_tensor("yout", [QLEN, C], fp32, kind="ExternalOutput")

    xin_t = xin.ap().rearrange("(t p) c -> p t c", p=128)     # [128, TT, C]
    yout_t = yout.ap().rearrange("(t p) c -> p t c", p=128)   # [128, QTT, C]

    with tile.TileContext(nc) as tc, contextlib.ExitStack() as ctx:
        per = ctx.enter_context(tc.tile_pool(name="per", bufs=1))
        dr = ctx.enter_context(tc.tile_pool(name="dr", bufs=2, space="DRAM"))
        ldx = ctx.enter_context(tc.tile_pool(name="ldx", bufs=6))
        xnp = ctx.enter_context(tc.tile_pool(name="xnp", bufs=6))
        expp = ctx.enter_context(tc.tile_pool(name="expp", bufs=6))
        rzp = ctx.enter_context(tc.tile_pool(name="rzp", bufs=3))
        ytp = ctx.enter_context(tc.tile_pool(name="ytp", bufs=4))
        hfp = ctx.enter_context(tc.tile_pool(name="hfp", bufs=2))
        sta = ctx.enter_context(tc.tile_pool(name="sta", bufs=1))
        # PSUM: 4 + 2 + 2 banks
        pss = ctx.enter_context(tc.tile_pool(name="pss", bufs=2, space="PSUM"))
        psa = ctx.enter_context(tc.tile_pool(name="psa", bufs=2, space="PSUM"))
        psm = ctx.enter_context(tc.tile_pool(name="psm", bufs=2, space="PSUM"))

        wqk = per.tile([128, CS, 768], bf16)
        nc.sync.dma_start(wqk[:], wqk_d.ap())
        wv = per.tile([128, CS, C], bf16)
        nc.sync.dma_start(wv[:], wv_d.ap())
        bias = per.tile([128, NBIAS], fp32)
        nc.sync.dma_start(bias[:], bias_d.ap())
        bv = per.tile([128, C], fp32)
        nc.sync.dma_start(bv[:], bv_d.ap().to_broadcast([128, C]))
        wp = per.tile([128, CS, C], bf16)
        wf1 = per.tile([128, CS, HID], bf16)
        wf2 = per.tile([128, KS, C], bf16)
        ident = per.tile([128, 128], bf16)
        make_identity(nc, ident)

        # PE warm-up burst: ~4.5us of back-to-back matmuls right after the
        # first weight DMA lands, so the HAM clock-gate opens (1.2->2.4GHz)
        # before the real (sparse) phase-A matmul stream begins.
        warm = psa.tile([128, NF], fp32, tag="aa", name="warm")
        for _ in range(20):
            nc.tensor.matmul(warm[:, :NF], wqk[:, 0, :128], wqk[:, 0, :NF],
                             start=True, stop=True)
        warmsink = per.tile([128, 1], fp32)
        nc.vector.tensor_copy(warmsink[:, 0:1], warm[:, 0:1])

        x_own = per.tile([128, QTT, C], fp32)
        x2 = per.tile([128, QTT, C], fp32)
        KT = per.tile([128, HP, SEQ], bf16)
        QT = per.tile([128, HP, QLEN], bf16)
        Vsb = per.tile([128, TT, H, VW], fp8)
        xnT = per.tile([128, CS, SEQ], bf16)
        xn2T = per.tile([128, CS, QLEN], bf16)
        AT = per.tile([128, HP, QLEN], bf16)

        nc.vector.memset(Vsb[:, :, :, DH], 1.0)   # Z ones column

        bv3 = bv.rearrange("p (hp x d) -> p hp x d", x=2, d=DH)
        v3 = Vsb.rearrange("p t (hp x) e -> p t hp x e", x=2)

        # ---------------- LN1 + transpose + V, per 4-tile group ----------------
        stats = sta.tile([128, TT, 8], fp32)   # nsum,sumsq,negmu,var,y,t,lnb,ex2

        def ln_stats_tile(xt, st):
            """per-tile reduction stats: negated sum + sum of squares."""
            nc.vector.tensor_reduce(
                st[:, 0:1], xt, axis=AX.X, op=ALU.add, negate=True)
            scr = ldx.tile([128, C], fp32, tag="scr", bufs=2)
            nc.vector.tensor_tensor(scr[:], xt, xt, ALU.mult)
            nc.vector.tensor_reduce(st[:, 1:2], scr[:], axis=AX.X, op=ALU.add)

        def ln_group_rstd(sg):
            """batched (group) rstd via DVE Newton: sg [128, G, 8].
            rstd -> col 4, lnb (=negmu*rstd) -> col 6."""
            nsum, sumsq = sg[:, :, 0], sg[:, :, 1]
            negmu, var = sg[:, :, 2], sg[:, :, 3]
            y, tmp, lnb_, ex2_ = sg[:, :, 4], sg[:, :, 5], sg[:, :, 6], sg[:, :, 7]
            nc.vector.tensor_scalar_mul(negmu, nsum, 1.0 / C)
            nc.vector.tensor_scalar_mul(ex2_, sumsq, 1.0 / C)
            nc.vector.tensor_tensor(var, negmu, negmu, ALU.mult)
            nc.vector.tensor_tensor(var, ex2_, var, ALU.subtract)
            nc.vector.tensor_scalar_add(var, var, EPS)
            # y0 = 1 folded into first Newton step: y1 = 1.5 - 0.5*v
            nc.vector.tensor_scalar(
                y, var, -0.5, 1.5, op0=ALU.mult, op1=ALU.add)
            for _ in range(2):
                nc.vector.tensor_tensor(tmp, y, y, ALU.mult)
                nc.vector.tensor_tensor(tmp, tmp, var, ALU.mult)
                nc.vector.tensor_scalar(
                    tmp, tmp, -0.5, 1.5, op0=ALU.mult, op1=ALU.add)
                nc.vector.tensor_tensor(y, y, tmp, ALU.mult)
            nc.vector.tensor_tensor(lnb_, negmu, y, ALU.mult)

        def ln_apply(xt, st, xn_out, on_act=False):
            if on_act:
                nc.scalar.activation(
                    xn_out, xt, AF.Identity, bias=st[:, 6:7], scale=st[:, 4:5])
            else:
                nc.vector.tensor_scalar(
                    xn_out, xt, st[:, 4:5], st[:, 6:7], op0=ALU.mult, op1=ALU.add)

        def transpose_to(xn, dstT, t, on_act=False):
            """3 PE transposes of xn [128, C] bf16 into dstT[:, :, t*128...]."""
            ptr = psm.tile([128, max(QF, CS * 128)], bf16, tag="sm", name="ptrA")
            for cs in range(CS):
                nc.tensor.transpose(
                    ptr[:, cs * 128:(cs + 1) * 128],
                    xn[:, cs * 128:(cs + 1) * 128], ident[:])
            src = ptr[:, :CS * 128].rearrange("p (cs n) -> p cs n", n=128)
            dst = dstT[:, :, t * 128:(t + 1) * 128]
            if on_act:
                nc.scalar.copy(dst, src)
            else:
                nc.vector.tensor_copy(dst, src)

        G = min(4, TT)
        xtiles = {}
        for g0 in range(0, TT, G):
            for t in range(g0, g0 + G):
                if t < QTT:
                    xt = x_own[:, t, :]
                else:
                    xts = ldx.tile([128, C], fp32, tag="xt")
                    xt = xts[:]
                xtiles[t] = xt
                nc.sync.dma_start(xt, xin_t[:, t, :])
                ln_stats_tile(xt, stats[:, t, :])
            ln_group_rstd(stats[:, g0:g0 + G, :])
            for t in range(g0, g0 + G):
                xn = xnp.tile([128, C], bf16, tag="xn")
                ln_apply(xtiles[t], stats[:, t, :], xn[:], on_act=True)
                transpose_to(xn, xnT, t, on_act=True)

                pv = psm.tile([128, max(QF, C)], fp32, tag="sm", name="pvA")
                for cs in range(CS):
                    nc.tensor.matmul(
                        pv[:, :C], xnT[:, cs, t * 128:(t + 1) * 128],
                        wv[:, cs, :],
                        start=(cs == 0), stop=(cs == CS - 1))
                pv3 = pv[:, :C].rearrange("p (hp x d) -> p hp x d", x=2, d=DH)
                for par in range(2):
                    nc.vector.tensor_tensor(
                        v3[:, t, :, par, :DH], pv3[:, :, par, :],
                        bv3[:, :, par, :], ALU.add)

        # deferred weight loads (needed only from proj/MLP onward) so the
        # x-tile loads win the DMA queues at startup
        nc.sync.dma_start(wp[:], wp_d.ap())
        nc.sync.dma_start(wf1[:], wf1_d.ap())
        nc.sync.dma_start(wf2[:], wf2_d.ap())

        def build_qk(m, evac_act=False):
            ntiles = NJ if m < HP else NN
            f = QF if m < HP else NF
            for n in range(ntiles):
                pk = psa.tile([128, NF], fp32, tag="aa", name="pkA")
                for cs in range(CS):
                    nc.tensor.matmul(
                        pk[:, :f],
                        wqk[:, cs, m * 128:(m + 1) * 128],
                        xnT[:, cs, n * f:(n + 1) * f],
                        start=(cs == 0), stop=(cs == CS - 1))
                if m < HP:
                    dst = QT[:, m, n * f:(n + 1) * f]
                else:
                    dst = KT[:, m - HP, n * f:(n + 1) * f]
                if evac_act:
                    nc.scalar.add(dst, pk[:, :f], bias[:, m:m + 1])
                else:
                    nc.vector.tensor_scalar_add(dst, pk[:, :f], bias[:, m:m + 1])

        # ---------------- attention ----------------
        def pv_pair(po, ech, k0, nk, h):
            # fp8 DoubleRow: one matmul contracts a PAIR of key tiles;
            # lhsT [128, 2, 65], rhs [128, 2, QF] -> out [65, QF]
            if nk == CK:
                nc.tensor.matmul(
                    po[:DH + 1, :], Vsb[:, k0:k0 + 2, h, :DH + 1], ech[:, :2, :],
                    start=(k0 == 0), stop=(k0 + 2 == TT),
                    perf_mode=mybir.MatmulPerfMode.DoubleRow)
            else:
                for i in range(nk):
                    kt = k0 + i
                    nc.tensor.matmul(
                        po[:DH + 1, :], Vsb[:, kt, h, :DH + 1], ech[:, i, :],
                        start=(kt == 0), stop=(kt == TT - 1))

        def attention_head(h, j):
            hp, hb = h // 2, (h % 2) * 64
            po = psm.tile([128, QF], fp32, tag="sm", name="po")
            echunks = []
            for ci, (k0, nk) in enumerate(chunks):
                psS = pss.tile([128, CK * QF], fp32, tag="ss")
                for i in range(nk):
                    kt = k0 + i
                    nc.tensor.matmul(
                        psS[:, i * QF:(i + 1) * QF],
                        KT[hb:hb + 64, hp, kt * 128:(kt + 1) * 128],
                        QT[hb:hb + 64, hp, j * QF:(j + 1) * QF],
                        start=True, stop=True)
                ech = expp.tile([128, CK, QF], fp8, tag="ech")
                nc.scalar.activation(
                    ech[:, :nk, :], psS[:, :nk * QF], AF.Exp)
                echunks.append((ech, k0, nk))
                if ci > 0:
                    pech, pk0, pnk = echunks[ci - 1]
                    pv_pair(po, pech, pk0, pnk, h)
            lech, lk0, lnk = echunks[-1]
            pv_pair(po, lech, lk0, lnk, h)

            # Z -> DRAM -> broadcast to 64 rows -> fast reciprocal -> mult
            rz = rzp.tile([128, QF], fp32, tag="rz")
            nc.vector.tensor_copy(rz[64:65, :], po[64:65, :])
            zscr = dr.tile([1, QF], fp32, tag="zscr")
            nc.sync.dma_start(zscr[:], rz[64:65, :])
            rzb = rzp.tile([64, QF], fp32, tag="rzb")
            nc.sync.dma_start(rzb[:], zscr.to_broadcast([64, QF]))
            rzr = rzp.tile([64, QF], fp32, tag="rzr")
            nc.vector.reciprocal_approx_fast(out=rzr[:], in_=rzb[:])
            nc.vector.tensor_tensor(
                AT[hb:hb + 64, hp, j * QF:(j + 1) * QF],
                po[:64, :], rzr[:], ALU.mult)

        def transpose_add(src_sb, dst, res):
            # src_sb [128, NB*128] bf16 -> transpose -> dst = res + src^T
            ptr = psm.tile([128, max(QF, CS * 128)], bf16, tag="sm", name="ptrC")
            for b in range(NB):
                nc.tensor.transpose(
                    ptr[:, b * 128:(b + 1) * 128],
                    src_sb[:, b * 128:(b + 1) * 128], ident[:])
            nc.vector.tensor_tensor(
                dst, ptr[:, :NB * 128].rearrange("p (b n) -> p b n", n=128),
                res, ALU.add)

        def proj_j(j):
            t0 = j * NB
            for m in range(CS):
                pp = psa.tile([128, NF], fp32, tag="aa", name="pp")
                for hp in range(HP):
                    nc.tensor.matmul(
                        pp[:, :QF], wp[:, hp, m * 128:(m + 1) * 128],
                        AT[:, hp, j * QF:(j + 1) * QF],
                        start=(hp == 0), stop=(hp == HP - 1))
                y1T = ytp.tile([128, QF], bf16, tag="yT")
                nc.vector.tensor_scalar_add(
                    y1T[:], pp[:, :QF], bias[:, 6 + m:7 + m])
                transpose_add(
                    y1T,
                    x2[:, t0:t0 + NB, m * 128:(m + 1) * 128],
                    x_own[:, t0:t0 + NB, m * 128:(m + 1) * 128])

        def ln2_j(j):
            t0 = j * NB
            for t in range(t0, t0 + NB):
                ln_stats_tile(x2[:, t, :], stats[:, t, :])
            ln_group_rstd(stats[:, t0:t0 + NB, :])
            for t in range(t0, t0 + NB):
                xn2 = xnp.tile([128, C], bf16, tag="xn")
                ln_apply(x2[:, t, :], stats[:, t, :], xn2[:])
                transpose_to(xn2, xn2T, t)

        def mlp_j(j):
            t0 = j * NB
            hful = hfp.tile([128, KS, QF], bf16, tag="hful")
            for ks in range(KS):
                pf1 = pss.tile([128, CK * QF], fp32, tag="ss", name="pf1")
                for cs in range(CS):
                    nc.tensor.matmul(
                        pf1[:, :QF], wf1[:, cs, ks * 128:(ks + 1) * 128],
                        xn2T[:, cs, j * QF:(j + 1) * QF],
                        start=(cs == 0), stop=(cs == CS - 1))
                nc.scalar.activation(
                    hful[:, ks, :], pf1[:, :QF], act_fn,
                    bias=bias[:, 9 + ks:10 + ks])
            for m in range(CS):
                pf2 = psa.tile([128, NF], fp32, tag="aa", name="pf2")
                for ks in range(KS):
                    nc.tensor.matmul(
                        pf2[:, :QF], wf2[:, ks, m * 128:(m + 1) * 128],
                        hful[:, ks, :],
                        start=(ks == 0), stop=(ks == KS - 1))
                y2T = ytp.tile([128, QF], bf16, tag="yT")
                nc.vector.tensor_scalar_add(
                    y2T[:], pf2[:, :QF], bias[:, 21 + m:22 + m])
                transpose_add(
                    y2T,
                    x2[:, t0:t0 + NB, m * 128:(m + 1) * 128],
                    x2[:, t0:t0 + NB, m * 128:(m + 1) * 128])
            nc.sync.dma_start(
                yout_t[:, t0:t0 + NB, :], x2[:, t0:t0 + NB, :])

        # interleave Q/K builds with attention per head pair; for the last
        # pair go j-major so proj(j) can start while j+1 attention runs
        for hpi in range(HP - 1):
            build_qk(hpi, evac_act=(hpi == 0))
            build_qk(HP + hpi, evac_act=(hpi == 0))
            for h in (2 * hpi, 2 * hpi + 1):
                for j in range(NJ):
                    attention_head(h, j)
        build_qk(HP - 1)
        build_qk(2 * HP - 1)
        for j in range(NJ):
            for h in (2 * HP - 2, 2 * HP - 1):
                attention_head(h, j)
            proj_j(j)
            ln2_j(j)
        for j in range(NJ):
            mlp_j(j)
    return nc


def prep_inputs(x, w_qkv, b_qkv, w_proj, b_proj, w_fc1, b_fc1, w_fc2, b_fc2,
                g1, beta1, g2, beta2, n_cores=8):
    """Host-side preprocessing: fold LN affine + attention scale into
    weights/biases, cast to bf16, reshape to SBUF layouts, permute x per core."""
    scale_q = DH ** -0.5

    wq = (g1[:, None] * w_qkv[:, :C]) * scale_q
    wk = g1[:, None] * w_qkv[:, C:2 * C]
    wv_ = g1[:, None] * w_qkv[:, 2 * C:]
    bq = (b_qkv[:C] + beta1 @ w_qkv[:, :C]) * scale_q
    bk = b_qkv[C:2 * C] + beta1 @ w_qkv[:, C:2 * C]
    bv_ = b_qkv[2 * C:] + beta1 @ w_qkv[:, 2 * C:]
    wf1_ = g2[:, None] * w_fc1
    bf1_ = b_fc1 + beta2 @ w_fc1

    def kx(w):
        n = w.shape[0] // 128
        return np.ascontiguousarray(
            w.reshape(n, 128, w.shape[1]).transpose(1, 0, 2)
        ).astype(ml_dtypes.bfloat16)

    wqk_l = kx(np.concatenate([wq, wk], axis=1))
    wv_l = kx(wv_)
    wp_l = kx(w_proj)

    wf1_l = kx(wf1_)
    wf2_l = kx(w_fc2)

    bias_h = np.zeros((128, NBIAS), np.float32)
    bias_h[:, 0:3] = bq.reshape(3, 128).T
    bias_h[:, 3:6] = bk.reshape(3, 128).T
    bias_h[:, 6:9] = b_proj.reshape(3, 128).T
    bias_h[:, 9:21] = bf1_.reshape(12, 128).T
    bias_h[:, 21:24] = b_fc2.reshape(3, 128).T
    bv_l = np.ascontiguousarray(bv_.reshape(1, C), dtype=np.float32)

    B, N, _ = x.shape
    half = N // 2
    in_maps = []
    for core in range(n_cores):
        b, hf = core // 2, core % 2
        own = x[b, hf * half:(hf + 1) * half]
        other = x[b, (1 - hf) * half:(2 - hf) * half]
        xin_core = np.ascontiguousarray(
            np.concatenate([own, other], axis=0), dtype=np.float32)
        in_maps.append({
            "xin": xin_core, "wqk": wqk_l, "wv": wv_l, "wp": wp_l,
            "wf1": wf1_l, "wf2": wf2_l, "bias": bias_h, "bv": bv_l,
        })
    return in_maps


def assemble_output(results, B, N):
    half = N // 2
    y = np.empty((B, N, C), np.float32)
    for core, r in enumerate(results):
        b, hf = core // 2, core % 2
        y[b, hf * half:(hf + 1) * half] = r["yout"]
    return y


_CACHED = {}


def _get_compiled(SEQ):
    if SEQ not in _CACHED:
        from concourse import bacc
        nc = bacc.Bacc("TRN2", target_bir_lowering=False, debug=False)
        build(nc, SEQ=SEQ)
        nc.compile()
        _CACHED[SEQ] = nc
    return _CACHED[SEQ]


def kernel(x, w_qkv, b_qkv, w_proj, b_proj, w_fc1, b_fc1, w_fc2, b_fc2,
           g1, beta1, g2, beta2):
    from concourse.bass_utils import run_bass_kernel_spmd

    x = np.asarray(x, dtype=np.float32)
    B, N, _ = x.shape
    nc = _get_compiled(N)
    in_maps = prep_inputs(
        x, np.asarray(w_qkv, np.float32), np.asarray(b_qkv, np.float32),
        np.asarray(w_proj, np.float32), np.asarray(b_proj, np.float32),
        np.asarray(w_fc1, np.float32), np.asarray(b_fc1, np.float32),
        np.asarray(w_fc2, np.float32), np.asarray(b_fc2, np.float32),
        np.asarray(g1, np.float32), np.asarray(beta1, np.float32),
        np.asarray(g2, np.float32), np.asarray(beta2, np.float32),
        n_cores=2 * B)
    res = run_bass_kernel_spmd(
        nc, in_maps, core_ids=list(range(2 * B)), trace=False)
    return assemble_output(res.results, B=B, N=N)



# revision 19
# speedup vs baseline: 1.0901x; 1.0901x over previous
"""TRN2 Bass/Tile kernel for nn_Block_19756849561899 (pre-LN transformer
block: LN -> MHA -> residual -> LN -> MLP(gelu) -> residual).

Self-contained: kernel(**inputs) takes the full fp32 tensors, shards work
across 8 NeuronCores (one batch per core-pair; each core owns half the
sequence as queries and redundantly builds K/V for its batch), compiles a
Bass/Tile program once per process, runs it SPMD, and reassembles the full
output.
"""

import contextlib

import numpy as np
import ml_dtypes

import concourse.bass as bass
import concourse.mybir as mybir
import concourse.tile as tile
from concourse.masks import make_identity

fp32 = mybir.dt.float32
bf16 = mybir.dt.bfloat16
fp8 = mybir.dt.float8e4
AF = mybir.ActivationFunctionType
ALU = mybir.AluOpType
AX = mybir.AxisListType

C = 384
CS = 3          # C / 128
H = 6
HP = 3          # head pairs
DH = 64
HID = 1536
KS = 12         # HID / 128
VW = 72         # padded V row width (DoubleRow needs 16B-aligned pair stride)
WSCALE = 16.0   # fp8 MLP weight scaling (avoids e4m3 denormals)
EPS = 1e-6
NBIAS = 24
BUST = "a3"     # bump on every IR change: the NEFF cache key misses
                # SBUF-internal IR edits, so the io signature must change


def build(nc, SEQ=2048, act_fn=AF.Gelu):
    TT = SEQ // 128          # token tiles over full sequence
    QTT = TT // 2            # token tiles in own (query) half
    QLEN = SEQ // 2
    QF = min(512, QLEN)      # q free-dim tile
    NJ = QLEN // QF
    NF = min(512, SEQ)       # seq free-dim tile for K^T build
    NN = SEQ // NF
    NB = QF // 128           # token blocks per q-tile
    CK = 2                   # key tiles per S/exp chunk
    chunks = [(k0, min(CK, TT - k0)) for k0 in range(0, TT, CK)]

    xin = nc.dram_tensor("xin", [SEQ, C], fp32, kind="ExternalInput")
    wqk_d = nc.dram_tensor("wqk", [128, CS, 768], bf16, kind="ExternalInput")
    wv_d = nc.dram_tensor("wv", [128, CS, C], bf16, kind="ExternalInput")
    wp_d = nc.dram_tensor("wp", [128, CS, C], bf16, kind="ExternalInput")
    wf1_d = nc.dram_tensor("wf1", [128, CS, HID], bf16, kind="ExternalInput")
    wf2_d = nc.dram_tensor("wf2", [128, KS, C], fp8, kind="ExternalInput")
    bias_d = nc.dram_tensor("bias", [128, NBIAS], fp32, kind="ExternalInput")
    bv_d = nc.dram_tensor("bv", [1, C], fp32, kind="ExternalInput")
    yout = nc.dram_tensor("yout", [QLEN, C], fp32, kind="ExternalOutput")
    bust_d = nc.dram_tensor(f"bustin_{BUST}", [1, 8], fp32, kind="ExternalInput")
    bust_o = nc.dram_tensor(f"bustout_{BUST}", [1, 8], fp32, kind="ExternalOutput")

    xin_t = xin.ap().rearrange("(t p) c -> p t c", p=128)     # [128, TT, C]
    yout_t = yout.ap().rearrange("(t p) c -> p t c", p=128)   # [128, QTT, C]

    with tile.TileContext(nc) as tc, contextlib.ExitStack() as ctx:
        per = ctx.enter_context(tc.tile_pool(name="per", bufs=1))
        dr = ctx.enter_context(tc.tile_pool(name="dr", bufs=2, space="DRAM"))
        ldx = ctx.enter_context(tc.tile_pool(name="ldx", bufs=6))
        xnp = ctx.enter_context(tc.tile_pool(name="xnp", bufs=6))
        expp = ctx.enter_context(tc.tile_pool(name="expp", bufs=6))
        rzp = ctx.enter_context(tc.tile_pool(name="rzp", bufs=3))
        ytp = ctx.enter_context(tc.tile_pool(name="ytp", bufs=4))
        hfp = ctx.enter_context(tc.tile_pool(name="hfp", bufs=2))
        sta = ctx.enter_context(tc.tile_pool(name="sta", bufs=1))
        # PSUM: 4 + 2 + 2 banks
        pss = ctx.enter_context(tc.tile_pool(name="pss", bufs=2, space="PSUM"))
        psa = ctx.enter_context(tc.tile_pool(name="psa", bufs=2, space="PSUM"))
        psm = ctx.enter_context(tc.tile_pool(name="psm", bufs=2, space="PSUM"))

        bust_t = per.tile([1, 8], fp32)
        nc.sync.dma_start(bust_t[:], bust_d.ap())
        nc.sync.dma_start(bust_o.ap(), bust_t[:])

        wqk = per.tile([128, CS, 768], bf16)
        nc.sync.dma_start(wqk[:], wqk_d.ap())
        wv = per.tile([128, CS, C], bf16)
        nc.sync.dma_start(wv[:], wv_d.ap())
        bias = per.tile([128, NBIAS], fp32)
        nc.sync.dma_start(bias[:], bias_d.ap())
        bv = per.tile([128, C], fp32)
        nc.sync.dma_start(bv[:], bv_d.ap().to_broadcast([128, C]))
        wp = per.tile([128, CS, C], bf16)
        wf1 = per.tile([128, CS, HID], bf16)
        wf2 = per.tile([128, KS, C], fp8)
        ident = per.tile([128, 128], bf16)
        make_identity(nc, ident)

        # PE warm-up burst: ~4.5us of back-to-back matmuls right after the
        # first weight DMA lands, so the HAM clock-gate opens (1.2->2.4GHz)
        # before the real (sparse) phase-A matmul stream begins.
        warm = psa.tile([128, NF], fp32, tag="aa", name="warm")
        for _ in range(20):
            nc.tensor.matmul(warm[:, :NF], wqk[:, 0, :128], wqk[:, 0, :NF],
                             start=True, stop=True)
        warmsink = per.tile([128, 1], fp32)
        nc.vector.tensor_copy(warmsink[:, 0:1], warm[:, 0:1])

        x_own = per.tile([128, QTT, C], fp32)
        x2 = per.tile([128, QTT, C], fp32)
        KT = per.tile([128, HP, SEQ], bf16)
        QT = per.tile([128, HP, QLEN], bf16)
        Vsb = per.tile([128, TT, H, VW], fp8)
        xnT = per.tile([128, CS, SEQ], bf16)
        xn2T = per.tile([128, CS, QLEN], bf16)
        AT = per.tile([128, HP, QLEN], bf16)

        nc.vector.memset(Vsb[:, :, :, DH], 1.0)   # Z ones column

        bv3 = bv.rearrange("p (hp x d) -> p hp x d", x=2, d=DH)
        v3 = Vsb.rearrange("p t (hp x) e -> p t hp x e", x=2)

        # ---------------- LN1 + transpose + V, per 4-tile group ----------------
        stats = sta.tile([128, TT, 8], fp32)   # _,_,mean,var,rstd,tmp,lnb,_

        def ln_stats_tile(xt, st):
            """per-tile mean/var via the fused BN_STATS path."""
            st6 = ldx.tile([128, 6], fp32, tag="st6", bufs=2)
            nc.vector.bn_stats(st6[:], xt)
            nc.vector.bn_aggr(st[:, 2:4], st6[:])

        def ln_group_rstd(sg):
            """batched (group) rstd via DVE Newton: sg [128, G, 8] with
            (mean, var) in cols 2,3. rstd -> col 4, lnb (=-mu*rstd) -> col 6."""
            mean, var = sg[:, :, 2], sg[:, :, 3]
            y, tmp, lnb_ = sg[:, :, 4], sg[:, :, 5], sg[:, :, 6]
            nc.vector.tensor_scalar_add(var, var, EPS)
            # y0 = 1 folded into first Newton step: y1 = 1.5 - 0.5*v
            nc.vector.tensor_scalar(
                y, var, -0.5, 1.5, op0=ALU.mult, op1=ALU.add)
            for _ in range(2):
                nc.vector.tensor_tensor(tmp, y, y, ALU.mult)
                nc.vector.tensor_tensor(tmp, tmp, var, ALU.mult)
                nc.vector.tensor_scalar(
                    tmp, tmp, -0.5, 1.5, op0=ALU.mult, op1=ALU.add)
                nc.vector.tensor_tensor(y, y, tmp, ALU.mult)
            nc.vector.tensor_tensor(tmp, mean, y, ALU.mult)
            nc.vector.tensor_scalar_mul(lnb_, tmp, -1.0)

        def ln_apply(xt, st, xn_out, on_act=False):
            if on_act:
                nc.scalar.activation(
                    xn_out, xt, AF.Identity, bias=st[:, 6:7], scale=st[:, 4:5])
            else:
                nc.vector.tensor_scalar(
                    xn_out, xt, st[:, 4:5], st[:, 6:7], op0=ALU.mult, op1=ALU.add)

        def transpose_to(xn, dstT, t, on_act=False):
            """3 PE transposes of xn [128, C] bf16 into dstT[:, :, t*128...]."""
            ptr = psm.tile([128, max(QF, CS * 128)], bf16, tag="sm", name="ptrA")
            for cs in range(CS):
                nc.tensor.transpose(
                    ptr[:, cs * 128:(cs + 1) * 128],
                    xn[:, cs * 128:(cs + 1) * 128], ident[:])
            src = ptr[:, :CS * 128].rearrange("p (cs n) -> p cs n", n=128)
            dst = dstT[:, :, t * 128:(t + 1) * 128]
            if on_act:
                nc.scalar.copy(dst, src)
            else:
                nc.vector.tensor_copy(dst, src)

        G = min(4, TT)
        xtiles = {}
        for g0 in range(0, TT, G):
            for t in range(g0, g0 + G):
                if t < QTT:
                    xt = x_own[:, t, :]
                else:
                    xts = ldx.tile([128, C], fp32, tag="xt")
                    xt = xts[:]
                xtiles[t] = xt
                nc.sync.dma_start(xt, xin_t[:, t, :])
                ln_stats_tile(xt, stats[:, t, :])
            ln_group_rstd(stats[:, g0:g0 + G, :])
            for t in range(g0, g0 + G):
                xn = xnp.tile([128, C], bf16, tag="xn")
                ln_apply(xtiles[t], stats[:, t, :], xn[:], on_act=True)
                transpose_to(xn, xnT, t, on_act=True)

                pv = psm.tile([128, max(QF, C)], fp32, tag="sm", name="pvA")
                for cs in range(CS):
                    nc.tensor.matmul(
                        pv[:, :C], xnT[:, cs, t * 128:(t + 1) * 128],
                        wv[:, cs, :],
                        start=(cs == 0), stop=(cs == CS - 1))
                pv3 = pv[:, :C].rearrange("p (hp x d) -> p hp x d", x=2, d=DH)
                for par in range(2):
                    nc.vector.tensor_tensor(
                        v3[:, t, :, par, :DH], pv3[:, :, par, :],
                        bv3[:, :, par, :], ALU.add)

        # deferred weight loads (needed only from proj/MLP onward) so the
        # x-tile loads win the DMA queues at startup
        nc.sync.dma_start(wp[:], wp_d.ap())
        nc.sync.dma_start(wf1[:], wf1_d.ap())
        nc.sync.dma_start(wf2[:], wf2_d.ap())

        def build_qk(m, evac_act=False):
            ntiles = NJ if m < HP else NN
            f = QF if m < HP else NF
            for n in range(ntiles):
                pk = psa.tile([128, NF], fp32, tag="aa", name="pkA")
                for cs in range(CS):
                    nc.tensor.matmul(
                        pk[:, :f],
                        wqk[:, cs, m * 128:(m + 1) * 128],
                        xnT[:, cs, n * f:(n + 1) * f],
                        start=(cs == 0), stop=(cs == CS - 1))
                if m < HP:
                    dst = QT[:, m, n * f:(n + 1) * f]
                else:
                    dst = KT[:, m - HP, n * f:(n + 1) * f]
                if evac_act:
                    nc.scalar.add(dst, pk[:, :f], bias[:, m:m + 1])
                else:
                    nc.vector.tensor_scalar_add(dst, pk[:, :f], bias[:, m:m + 1])

        # ---------------- attention ----------------
        def pv_pair(po, ech, k0, nk, h):
            # fp8 DoubleRow: one matmul contracts a PAIR of key tiles;
            # lhsT [128, 2, 65], rhs [128, 2, QF] -> out [65, QF]
            if nk == CK:
                nc.tensor.matmul(
                    po[:DH + 1, :], Vsb[:, k0:k0 + 2, h, :DH + 1], ech[:, :2, :],
                    start=(k0 == 0), stop=(k0 + 2 == TT),
                    perf_mode=mybir.MatmulPerfMode.DoubleRow)
            else:
                for i in range(nk):
                    kt = k0 + i
                    nc.tensor.matmul(
                        po[:DH + 1, :], Vsb[:, kt, h, :DH + 1], ech[:, i, :],
                        start=(kt == 0), stop=(kt == TT - 1))

        def attention_head(h, j):
            hp, hb = h // 2, (h % 2) * 64
            po = psm.tile([128, QF], fp32, tag="sm", name="po")
            echunks = []
            for ci, (k0, nk) in enumerate(chunks):
                psS = pss.tile([128, CK * QF], fp32, tag="ss")
                for i in range(nk):
                    kt = k0 + i
                    nc.tensor.matmul(
                        psS[:, i * QF:(i + 1) * QF],
                        KT[hb:hb + 64, hp, kt * 128:(kt + 1) * 128],
                        QT[hb:hb + 64, hp, j * QF:(j + 1) * QF],
                        start=True, stop=True)
                ech = expp.tile([128, CK, QF], fp8, tag="ech")
                nc.scalar.activation(
                    ech[:, :nk, :], psS[:, :nk * QF], AF.Exp)
                echunks.append((ech, k0, nk))
                if ci > 0:
                    pech, pk0, pnk = echunks[ci - 1]
                    pv_pair(po, pech, pk0, pnk, h)
            lech, lk0, lnk = echunks[-1]
            pv_pair(po, lech, lk0, lnk, h)

            # Z row (PSUM lane 64) -> SBUF lane 0 -> fast reciprocal ->
            # gpsimd partition-broadcast -> normalize-evacuate to AT.
            # NOTE: custom-DVE ops and partition_broadcast only work at
            # partition base 0; the standard copy does the lane shift.
            z0 = rzp.tile([1, QF], fp32, tag="rz")
            nc.vector.tensor_copy(z0[:], po[64:65, :])
            rzr = rzp.tile([1, QF], fp32, tag="rzr")
            nc.vector.reciprocal_approx_fast(out=rzr[:], in_=z0[:])
            rzb = rzp.tile([64, QF], fp32, tag="rzb")
            nc.gpsimd.partition_broadcast(rzb[:], rzr[:])
            nc.vector.tensor_tensor(
                AT[hb:hb + 64, hp, j * QF:(j + 1) * QF],
                po[:64, :], rzb[:], ALU.mult)

        def transpose_add(src_sb, dst, res):
            # src_sb [128, NB*128] bf16 -> transpose -> dst = res + src^T
            ptr = psm.tile([128, max(QF, CS * 128)], bf16, tag="sm", name="ptrC")
            for b in range(NB):
                nc.tensor.transpose(
                    ptr[:, b * 128:(b + 1) * 128],
                    src_sb[:, b * 128:(b + 1) * 128], ident[:])
            nc.vector.tensor_tensor(
                dst, ptr[:, :NB * 128].rearrange("p (b n) -> p b n", n=128),
                res, ALU.add)

        def proj_j(j):
            t0 = j * NB
            for m in range(CS):
                pp = psa.tile([128, NF], fp32, tag="aa", name="pp")
                for hp in range(HP):
                    nc.tensor.matmul(
                        pp[:, :QF], wp[:, hp, m * 128:(m + 1) * 128],
                        AT[:, hp, j * QF:(j + 1) * QF],
                        start=(hp == 0), stop=(hp == HP - 1))
                y1T = ytp.tile([128, QF], bf16, tag="yT")
                nc.vector.tensor_scalar_add(
                    y1T[:], pp[:, :QF], bias[:, 6 + m:7 + m])
                transpose_add(
                    y1T,
                    x2[:, t0:t0 + NB, m * 128:(m + 1) * 128],
                    x_own[:, t0:t0 + NB, m * 128:(m + 1) * 128])

        def ln2_j(j):
            t0 = j * NB
            for t in range(t0, t0 + NB):
                ln_stats_tile(x2[:, t, :], stats[:, t, :])
            ln_group_rstd(stats[:, t0:t0 + NB, :])
            for t in range(t0, t0 + NB):
                xn2 = xnp.tile([128, C], bf16, tag="xn")
                ln_apply(x2[:, t, :], stats[:, t, :], xn2[:])
                transpose_to(xn2, xn2T, t)

        def mlp_j(j):
            t0 = j * NB
            hful = hfp.tile([128, KS, QF], fp8, tag="hful")
            for ks in range(KS):
                pf1 = pss.tile([128, CK * QF], fp32, tag="ss", name="pf1")
                for cs in range(CS):
                    nc.tensor.matmul(
                        pf1[:, :QF], wf1[:, cs, ks * 128:(ks + 1) * 128],
                        xn2T[:, cs, j * QF:(j + 1) * QF],
                        start=(cs == 0), stop=(cs == CS - 1))
                nc.scalar.activation(
                    hful[:, ks, :], pf1[:, :QF], act_fn,
                    bias=bias[:, 9 + ks:10 + ks])
            for m in range(CS):
                pf2 = psa.tile([128, NF], fp32, tag="aa", name="pf2")
                for k2 in range(KS // 2):
                    nc.tensor.matmul(
                        pf2[:, :QF], wf2[:, 2 * k2:2 * k2 + 2, m * 128:(m + 1) * 128],
                        hful[:, 2 * k2:2 * k2 + 2, :],
                        start=(k2 == 0), stop=(k2 == KS // 2 - 1),
                        perf_mode=mybir.MatmulPerfMode.DoubleRow)
                y2T = ytp.tile([128, QF], bf16, tag="yT")
                nc.vector.tensor_scalar(
                    y2T[:], pf2[:, :QF], 1.0 / WSCALE, bias[:, 21 + m:22 + m],
                    op0=ALU.mult, op1=ALU.add)
                transpose_add(
                    y2T,
                    x2[:, t0:t0 + NB, m * 128:(m + 1) * 128],
                    x2[:, t0:t0 + NB, m * 128:(m + 1) * 128])
            nc.sync.dma_start(
                yout_t[:, t0:t0 + NB, :], x2[:, t0:t0 + NB, :])

        # interleave Q/K builds with attention per head pair; for the last
        # pair go j-major so proj(j) can start while j+1 attention runs
        for hpi in range(HP - 1):
            build_qk(hpi, evac_act=(hpi == 0))
            build_qk(HP + hpi, evac_act=(hpi == 0))
            for h in (2 * hpi, 2 * hpi + 1):
                for j in range(NJ):
                    attention_head(h, j)
        build_qk(HP - 1)
        build_qk(2 * HP - 1)
        for j in range(NJ):
            for h in (2 * HP - 2, 2 * HP - 1):
                attention_head(h, j)
            proj_j(j)
            ln2_j(j)
        for j in range(NJ):
            mlp_j(j)
    return nc


def prep_inputs(x, w_qkv, b_qkv, w_proj, b_proj, w_fc1, b_fc1, w_fc2, b_fc2,
                g1, beta1, g2, beta2, n_cores=8):
    """Host-side preprocessing: fold LN affine + attention scale into
    weights/biases, cast to bf16, reshape to SBUF layouts, permute x per core."""
    scale_q = DH ** -0.5

    wq = (g1[:, None] * w_qkv[:, :C]) * scale_q
    wk = g1[:, None] * w_qkv[:, C:2 * C]
    wv_ = g1[:, None] * w_qkv[:, 2 * C:]
    bq = (b_qkv[:C] + beta1 @ w_qkv[:, :C]) * scale_q
    bk = b_qkv[C:2 * C] + beta1 @ w_qkv[:, C:2 * C]
    bv_ = b_qkv[2 * C:] + beta1 @ w_qkv[:, 2 * C:]
    wf1_ = g2[:, None] * w_fc1
    bf1_ = b_fc1 + beta2 @ w_fc1

    def kx(w):
        n = w.shape[0] // 128
        return np.ascontiguousarray(
            w.reshape(n, 128, w.shape[1]).transpose(1, 0, 2)
        ).astype(ml_dtypes.bfloat16)

    wqk_l = kx(np.concatenate([wq, wk], axis=1))
    wv_l = kx(wv_)
    wp_l = kx(w_proj)

    wf1_l = kx(wf1_)

    def kx8(w):
        n = w.shape[0] // 128
        return np.ascontiguousarray(
            w.reshape(n, 128, w.shape[1]).transpose(1, 0, 2)
        ).astype(ml_dtypes.float8_e4m3)

    def_kx8 = None

    def kx8(w):
        n = w.shape[0] // 128
        return np.ascontiguousarray(
            w.reshape(n, 128, w.shape[1]).transpose(1, 0, 2)
        ).astype(ml_dtypes.float8_e4m3)

    wf2_l = kx8(w_fc2 * WSCALE)

    bias_h = np.zeros((128, NBIAS), np.float32)
    bias_h[:, 0:3] = bq.reshape(3, 128).T
    bias_h[:, 3:6] = bk.reshape(3, 128).T
    bias_h[:, 6:9] = b_proj.reshape(3, 128).T
    bias_h[:, 9:21] = bf1_.reshape(12, 128).T
    bias_h[:, 21:24] = b_fc2.reshape(3, 128).T
    bv_l = np.ascontiguousarray(bv_.reshape(1, C), dtype=np.float32)

    B, N, _ = x.shape
    half = N // 2
    in_maps = []
    for core in range(n_cores):
        b, hf = core // 2, core % 2
        own = x[b, hf * half:(hf + 1) * half]
        other = x[b, (1 - hf) * half:(2 - hf) * half]
        xin_core = np.ascontiguousarray(
            np.concatenate([own, other], axis=0), dtype=np.float32)
        in_maps.append({
            "xin": xin_core, "wqk": wqk_l, "wv": wv_l, "wp": wp_l,
            "wf1": wf1_l, "wf2": wf2_l, "bias": bias_h, "bv": bv_l,
            f"bustin_{BUST}": np.zeros((1, 8), np.float32),
        })
    return in_maps


def assemble_output(results, B, N):
    half = N // 2
    y = np.empty((B, N, C), np.float32)
    for core, r in enumerate(results):
        b, hf = core // 2, core % 2
        y[b, hf * half:(hf + 1) * half] = r["yout"]
    return y


_CACHED = {}


def _get_compiled(SEQ):
    if SEQ not in _CACHED:
        from concourse import bacc
        nc = bacc.Bacc("TRN2", target_bir_lowering=False, debug=False)
        build(nc, SEQ=SEQ)
        nc.compile()
        _CACHED[SEQ] = nc
    return _CACHED[SEQ]


def kernel(x, w_qkv, b_qkv, w_proj, b_proj, w_fc1, b_fc1, w_fc2, b_fc2,
           g1, beta1, g2, beta2):
    from concourse.bass_utils import run_bass_kernel_spmd

    x = np.asarray(x, dtype=np.float32)
    B, N, _ = x.shape
    nc = _get_compiled(N)
    in_maps = prep_inputs(
        x, np.asarray(w_qkv, np.float32), np.asarray(b_qkv, np.float32),
        np.asarray(w_proj, np.float32), np.asarray(b_proj, np.float32),
        np.asarray(w_fc1, np.float32), np.asarray(b_fc1, np.float32),
        np.asarray(w_fc2, np.float32), np.asarray(b_fc2, np.float32),
        np.asarray(g1, np.float32), np.asarray(beta1, np.float32),
        np.asarray(g2, np.float32), np.asarray(beta2, np.float32),
        n_cores=2 * B)
    res = run_bass_kernel_spmd(
        nc, in_maps, core_ids=list(range(2 * B)), trace=False)
    return assemble_output(res.results, B=B, N=N)

